# revision 15
# baseline (speedup 1.0000x reference)
"""Distributed TRN2 kernel for nn_Att_scores (attention score double-sum).

Math: reference computes
    qkv = X @ W_qkv.T ; q, k = split ; attn = (q @ k^T) * scale
    scores = attn.sum(heads).sum(keys)                      # [B, N]
Since the head/key sums commute with the matmuls, this is exactly
    Xsum[b]  = sum_n X[b, n, :]                             # [C]
    t[b]     = Wk @ Xsum[b]        (Wk = W_qkv[C:2C])       # [C]  (= sum_m k[b,m])
    u[b]     = Wq^T @ t[b]         (Wq = W_qkv[0:C])        # [C]
    scores[b, n] = scale * X[b, n, :] . u[b]
one global row-sum, two 768x768 matvecs, and one per-row dot.

Distribution: shard N across the 8 cores (each core owns 256 rows of both
batches).  Each core computes its partial Xsum on TensorE (ones-vector
matmul over natural-layout X tiles); a 6 KB AllReduce materialises the full
Xsum on every core; every core then computes t/u redundantly and the final
dot for its own rows.

Engine mapping:
  * partial Xsum:  TensorE (lhsT = ones column)
  * Wk^T:          TensorE transpose-mode, overlapped with the AllReduce
  * t = Wk@Xsum:   TensorE (lhsT = Xsum^T chunks, rhs = Wk^T chunks)
  * u = Wq^T@t:    TensorE (lhsT = t columns, rhs = natural-layout Wq)
  * dot:           VectorE tensor_mul against a TensorE-broadcast of u,
                   reduced along the free axis by ScalarE's fused
                   activation accumulate (runs concurrently with VectorE)
"""

import numpy as np

B = 2
N = 2048
C = 768
H = 12
HD = C // H
SCALE = float(HD) ** -0.5
NCORES = 8
NS = N // NCORES          # 256 rows of each batch per core
CH = NS // 128            # 2 partition-chunks per batch per core
JT = C // 128             # 6 128-row tiles of each W half

_compiled_nc = None


def _build_and_compile(use_collective=True):
    import concourse.bass as bass  # noqa: F401
    import concourse.bacc as bacc
    import concourse.tile as tile
    import concourse.mybir as mybir
    from concourse import masks

    f32 = mybir.dt.float32
    add = mybir.AluOpType.add
    mult = mybir.AluOpType.mult
    copy_fn = mybir.ActivationFunctionType.Copy

    nc = bacc.Bacc(
        "TRN2",
        target_bir_lowering=False,
        debug=False,
        num_devices=NCORES,
    )

    x_d = nc.dram_tensor("x_in", [B, NS, C], f32, kind="ExternalInput")
    w_d = nc.dram_tensor("w_in", [2 * C, C], f32, kind="ExternalInput")
    out_d = nc.dram_tensor("scores_out", [B, NS], f32, kind="ExternalOutput")

    # PSUM-bank-safe free-dim slices (2 KB fp32 bank = 512 elements)
    SLICES = ((0, 512), (512, 256))

    with tile.TileContext(nc) as tc:
        with (
            tc.tile_pool(name="sbuf", bufs=1) as pool,
            tc.tile_pool(name="psum", bufs=1, space="PSUM") as psum,
            tc.tile_pool(name="dram", bufs=1, space="DRAM") as dram,
        ):
            # ---------------- SBUF residents ----------------
            x_sb = pool.tile([128, B * CH, C], f32)    # [p, (b,ch), c]
            wk_sb = pool.tile([128, JT, C], f32)       # Wk row j = jt*128+p
            wq_sb = pool.tile([128, JT, C], f32)       # Wq row j = jt*128+p
            wkT_sb = pool.tile([128, JT, C], f32)      # Wk^T: [c-part, ck, j]
            ones_col = pool.tile([1, 128], f32)        # lhsT for partition-bcast
            ones_red = pool.tile([128, 1], f32)        # lhsT for row-sum
            ident = pool.tile([128, 128], f32)         # transpose identity
            xsp_sb = pool.tile([1, B, C], f32)         # partial Xsum rows
            xsum_sb = pool.tile([B, C], f32)           # full Xsum, b on partitions
            xsumT_sb = pool.tile([128, JT, B], f32)    # Xsum^T: [c-part, ck, b]
            t2_sb = pool.tile([B, C], f32)             # t^T: [b-part, j]
            t_sb = pool.tile([128, JT, B], f32)        # t: [j-part, ck, b]
            u_sb = pool.tile([1, B, C], f32)           # u rows (partition 0)
            prod_sb = pool.tile([128, C], f32)         # X*u elementwise scratch
            junk_sb = pool.tile([128, C], f32)         # ACT mandatory out
            sc_sb = pool.tile([128, B, CH], f32)       # scores[p, b, ch]

            # ---------------- loads (dependency order) ----------------
            nc.sync.dma_start(
                x_sb[:], x_d.ap().rearrange("b (ch p) c -> p (b ch) c", p=128)
            )
            nc.sync.dma_start(
                wk_sb[:], w_d[C : 2 * C, :].rearrange("(t p) c -> p t c", p=128)
            )
            nc.sync.dma_start(
                wq_sb[:], w_d[0:C, :].rearrange("(t p) c -> p t c", p=128)
            )
            nc.gpsimd.memset(ones_col[:], 1.0)
            nc.gpsimd.memset(ones_red[:], 1.0)
            masks.make_identity(nc, ident[:])

            # ---------------- partial Xsum (TensorE) ----------------
            # out[1, c] = ones[128n, 1].T @ X_tile[128n, c], PSUM-accumulated
            # over the two 128-row chunks.
            for b in range(B):
                xs_ps = psum.tile([1, 1024], f32, tag="small", bufs=1, name=f"xs{b}")
                for lo, nsz in SLICES:
                    for ch in range(CH):
                        nc.tensor.matmul(
                            xs_ps[:, lo : lo + nsz],
                            ones_red[:],
                            x_sb[:, b * CH + ch, lo : lo + nsz],
                            start=(ch == 0),
                            stop=(ch == CH - 1),
                        )
                    nc.vector.tensor_copy(
                        xsp_sb[:, b, lo : lo + nsz], xs_ps[:, lo : lo + nsz]
                    )

            # ---------------- AllReduce of [B, C] partial sums ----------------
            ar_in = dram.tile([1, B, C], f32)
            ar_out = dram.tile([1, B, C], f32)
            nc.sync.dma_start(ar_in[:], xsp_sb[:])
            if use_collective:
                nc.gpsimd.collective_compute(
                    "AllReduce",
                    add,
                    replica_groups=[list(range(NCORES))],
                    ins=[ar_in.opt()],
                    outs=[ar_out.opt()],
                )
            else:
                nc.sync.dma_start(ar_out[:], ar_in[:])
            # land with b on partitions for the transposes below
            nc.sync.dma_start(xsum_sb[:], ar_out[:].rearrange("x b c -> (x b) c"))

            # ---------------- Wk^T via TensorE transpose ----------------
            # Independent of the collective: runs during the AllReduce wait.
            for jt in range(JT):
                for ck in range(JT):
                    tr_ps = psum.tile(
                        [128, 128], f32, tag="tr", bufs=2, name=f"wt{jt}_{ck}"
                    )
                    nc.tensor.transpose(
                        tr_ps[:],
                        wk_sb[:, jt, ck * 128 : (ck + 1) * 128],
                        ident[:],
                    )
                    nc.vector.tensor_copy(
                        wkT_sb[:, ck, jt * 128 : (jt + 1) * 128], tr_ps[:]
                    )

            # ---------------- Xsum^T via TensorE transpose ----------------
            for ck in range(JT):
                xt_ps = psum.tile([128, 2], f32, tag="tr", bufs=2, name=f"xt{ck}")
                nc.tensor.transpose(
                    xt_ps[:],
                    xsum_sb[:, ck * 128 : (ck + 1) * 128],
                    ident[0:B, 0:B],
                )
                nc.scalar.copy(xsumT_sb[:, ck, :], xt_ps[:])

            # ---------------- t^T[b, j] = sum_c Xsum^T[c,b] * Wk^T[c,j] ----
            tt_ps = psum.tile([B, 1024], f32, tag="small", bufs=1)
            for lo, nsz in SLICES:
                for ck in range(JT):
                    nc.tensor.matmul(
                        tt_ps[:, lo : lo + nsz],
                        xsumT_sb[:, ck, :],
                        wkT_sb[:, ck, lo : lo + nsz],
                        start=(ck == 0),
                        stop=(ck == JT - 1),
                    )
                nc.vector.tensor_copy(t2_sb[:, lo : lo + nsz], tt_ps[:, lo : lo + nsz])

            # ---------------- t = (t^T)^T via TensorE transpose ----------
            for ck in range(JT):
                ts_ps = psum.tile([128, 2], f32, tag="tr", bufs=2, name=f"ts{ck}")
                nc.tensor.transpose(
                    ts_ps[:],
                    t2_sb[:, ck * 128 : (ck + 1) * 128],
                    ident[0:B, 0:B],
                )
                nc.scalar.copy(t_sb[:, ck, :], ts_ps[:])

            # ---------------- u[b, c'] = sum_j Wq[j, c'] * t[j, b] ----------
            for b in range(B):
                u_ps = psum.tile([1, 1024], f32, tag="mid", bufs=1, name=f"u{b}")
                for lo, nsz in SLICES:
                    for ck in range(JT):
                        nc.tensor.matmul(
                            u_ps[:, lo : lo + nsz],
                            t_sb[:, ck, b : b + 1],
                            wq_sb[:, ck, lo : lo + nsz],
                            start=(ck == 0),
                            stop=(ck == JT - 1),
                        )
                    nc.scalar.copy(u_sb[:, b, lo : lo + nsz], u_ps[:, lo : lo + nsz])

            # ---------------- scores = scale * <X, ones x u[b]> -------------
            for b in range(B):
                ub_ps = psum.tile([128, 1024], f32, tag="mid", bufs=1, name=f"ub{b}")
                for lo, nsz in SLICES:
                    nc.tensor.matmul(
                        ub_ps[:, lo : lo + nsz],
                        ones_col[:],
                        u_sb[:, b, lo : lo + nsz],
                    )
                for ch in range(CH):
                    for lo, nsz in SLICES:
                        nc.vector.tensor_mul(
                            prod_sb[:, lo : lo + nsz],
                            x_sb[:, b * CH + ch, lo : lo + nsz],
                            ub_ps[:, lo : lo + nsz],
                        )
                    nc.scalar.activation(
                        junk_sb[:],
                        prod_sb[:],
                        copy_fn,
                        scale=SCALE,
                        accum_out=sc_sb[:, b, ch : ch + 1],
                    )

            # ---------------- store ----------------
            nc.sync.dma_start(
                out_d.ap().rearrange("b (ch p) -> p b ch", p=128), sc_sb[:]
            )

    nc.compile()
    return nc


def _get_nc():
    global _compiled_nc
    if _compiled_nc is None:
        _compiled_nc = _build_and_compile()
    return _compiled_nc


def make_in_maps(X, W_qkv):
    X = np.ascontiguousarray(X, dtype=np.float32)
    W = np.ascontiguousarray(W_qkv, dtype=np.float32)
    assert X.shape == (B, N, C) and W.shape == (2 * C, C)
    return [
        {"x_in": np.ascontiguousarray(X[:, i * NS : (i + 1) * NS, :]), "w_in": W}
        for i in range(NCORES)
    ]


def assemble_out(results):
    return np.concatenate(
        [results[i]["scores_out"] for i in range(NCORES)], axis=1
    ).astype(np.float32)


def kernel(X, W_qkv):
    from concourse import bass_utils

    nc = _get_nc()
    res = bass_utils.run_bass_kernel_spmd(
        nc, make_in_maps(X, W_qkv), core_ids=list(range(NCORES))
    )
    return assemble_out(res.results)
